# revision 17
# baseline (speedup 1.0000x reference)
"""Trainium2 Bass kernel for nn_MultiHeadAttention_73624329388477.

Reference computation (B=8, S=1024, D=1024, H=16, depth=64):
    qh = split_heads(BN1(relu(q @ wq + bq)))      (BN inference: affine)
    kh = split_heads(BN2(relu(k @ wk + bk)))
    vh = split_heads(BN3(relu(v @ wv + bv)))
    scores = qh @ kh^T / sqrt(64) + mask * -1e9
    attn = softmax(scores, axis=-1)
    ctx = attn @ vh  -> concat heads
    out = relu(ctx @ wo + bo)
    returns (out, attn)

Sharding: data-parallel over batch. Core b computes batch b entirely
(full projections + 16 heads + output projection). No collectives.

Per-core layout strategy (all matmuls in float32r: 1 cycle/row on PE):
  - q,k,v arrive HOST-pre-transposed as [channel, seq] so the in-channel
    contraction sits on the partition dim with no on-chip transposes.
  - Q/K/V projections produce qpT/kpT/vpT in [out_ch, seq] layout
    (lhsT = weight k-strip, rhs = xT). ReLU+bias fused on ScalarE
    (per-partition bias), BN affine fused on VectorE tensor_scalar
    (per-partition scale+shift). The 1/sqrt(depth) factor is folded
    into the q-path BN scale on the host.
  - attention per head computes scoresT [k, q] = kh @ qh^T. The mask is
    per-k = per-partition, so exp(scores + mask*-1e9) is ONE ScalarE
    activation straight out of PSUM with the mask column as bias.
  - V is PE-transposed once into vp_aug [seq, 16*(64+1)]: per head 64
    value columns plus a ones column. The ctx matmul
    matmul(lhsT=vp_aug_head[k, 65], rhs=e^T[k, q]) then yields ctxT'
    rows 0:64 AND the softmax denominators in row 64 for free.
  - the denominator row bounces through DRAM and broadcast-loads
    (0-step partition AP) to [128, q]; reciprocal runs full-width.
  - e^T is normalized in place (split across VectorE and GpSimd) and
    DMA'd out as attn^T (host returns a transposed view); ctxT' is
    normalized during PSUM eviction by the same inv tile.
  - output projection in [out_ch, seq] layout (lhsT=wo strip,
    rhs=ctxT) with per-partition bias + ReLU; host transposes the view.
  - qpT/kpT/ctxT round-trip through DRAM scratch to stay inside SBUF.
"""

import numpy as np

import concourse.bass as bass
import concourse.bacc as bacc
import concourse.tile as tile
from concourse import mybir
from concourse.bass_utils import run_bass_kernel_spmd

S = 1024
D = 1024
H = 16
DP = 64
P = 128
NCORES = 8
BN_EPS = 1e-3

FR = mybir.dt.float32r
F32 = mybir.dt.float32

ST = S // P      # 8 seq tiles
CT = D // P      # 8 channel tiles
NH = S // 512    # 2 free-dim halves of 512
HB = DP + 1      # head block in vp_aug: 64 value cols + ones col

AF = mybir.ActivationFunctionType
OP = mybir.AluOpType


def build_nc():
    # Bacc (not plain Bass): its compile() pass splits multi-semaphore
    # waits into chains — TPB instructions only take ONE sync wait each.
    nc = bacc.Bacc(None, target_bir_lowering=False)

    xq = nc.dram_tensor("xq", [D, S], FR, kind="ExternalInput")  # q[b].T
    xk = nc.dram_tensor("xk", [D, S], FR, kind="ExternalInput")  # k[b].T
    xv = nc.dram_tensor("xv", [D, S], FR, kind="ExternalInput")  # v[b].T
    wq = nc.dram_tensor("wq", [D, D], FR, kind="ExternalInput")
    wk = nc.dram_tensor("wk", [D, D], FR, kind="ExternalInput")
    wv = nc.dram_tensor("wv", [D, D], FR, kind="ExternalInput")
    wo = nc.dram_tensor("wo", [D, D], FR, kind="ExternalInput")
    # params[p, j*8+m] = vec_j[m*128+p]; j: bq,sq,tq,bk,sk,tk,bv,sv,tv,bo
    params = nc.dram_tensor("params", [P, 10 * CT], F32, kind="ExternalInput")
    # msk[p, t] = -1e9 * mask[t*128+p]
    msk = nc.dram_tensor("msk", [P, ST], F32, kind="ExternalInput")
    # [eye(128) | ones(128,16)] — walrus rejects memset of float32r
    # tiles, so the transpose identity and ones columns arrive via DMA.
    consts = nc.dram_tensor("consts", [P, P + H], FR, kind="ExternalInput")

    outT = nc.dram_tensor("outT", [D, S], FR, kind="ExternalOutput")
    attnT = nc.dram_tensor("attnT", [H, S, S], FR, kind="ExternalOutput")

    # DRAM scratch (SBUF can't hold qpT/kpT/ctxT for the whole kernel)
    qpT_sc = nc.dram_tensor("qpT_sc", [D, S], FR)
    kpT_sc = nc.dram_tensor("kpT_sc", [D, S], FR)
    ctx_sc = nc.dram_tensor("ctx_sc", [D, S], FR)
    sum_sc = nc.dram_tensor("sum_sc", [H, S], FR)
    inv_sc = nc.dram_tensor("inv_sc", [H, S], FR)

    with tile.TileContext(nc) as tc:
        with (
            # float32r tiles are bit-identical fp32 storage; only the PE
            # multiply path is reduced-precision, so f32r outputs from
            # DVE/ACT ops lose nothing.
            nc.allow_low_precision(reason="float32r storage is fp32"),
            tc.tile_pool(name="res", bufs=1) as res,
            tc.tile_pool(name="ring", bufs=26) as ring,
            tc.tile_pool(name="stage", bufs=4) as stage,
            tc.tile_pool(name="cstage", bufs=3) as cstage,
            tc.tile_pool(name="bcast", bufs=3) as bcast,
            tc.tile_pool(name="rows", bufs=2) as rows,
            tc.tile_pool(name="pmm", bufs=2, space="PSUM") as pmm,
            tc.tile_pool(name="pctx", bufs=2, space="PSUM") as pctx,
            tc.tile_pool(name="ptr", bufs=2, space="PSUM") as ptr,
        ):
            cst_in = res.tile([P, P + H], FR, name="cst_in")
            nc.sync.dma_start(cst_in, consts[:, :])
            ident = cst_in[:, :P]
            prm = res.tile([P, 10 * CT], F32, name="prm")
            nc.sync.dma_start(prm, params[:, :])
            mcol = res.tile([P, ST], F32, name="mcol")
            nc.sync.dma_start(mcol, msk[:, :])

            def pcol(j, m):  # [128,1] per-partition column of param j
                return prm[:, j * CT + m : j * CT + m + 1]

            # vp_aug[st][:, h*65 : h*65+64] = v-projection head h rows;
            # col h*65+64 = ones (yields softmax sums in the ctx matmul).
            vpa = [res.tile([P, H * HB], FR, name=f"vpa{i}") for i in range(ST)]
            # Let PE observe ident's DMA once so later transposes carry
            # only their own input's wait (one sync wait per TPB inst).
            pwarm = ptr.tile([P, P], FR, tag="tr", name="pwarm")
            nc.tensor.transpose(pwarm, ident, ident)
            for st in range(ST):
                # ones columns, written once via a strided DMA
                ones_dst = vpa[st].rearrange(
                    "p (h c) -> p h c", c=HB
                )[:, :, DP : DP + 1]
                nc.sync.dma_start(ones_dst, consts[:, P : P + H])

            # ---- phase A: projections ----
            def project(xT_dram, w_dram, jb, js, jt, out_tiles, out_dram):
                xT = [
                    ring.tile([P, S], FR, tag="ring", name=f"xT{i}")
                    for i in range(CT)
                ]
                ws = [
                    ring.tile([P, D], FR, tag="ring", name=f"w{i}")
                    for i in range(CT)
                ]
                for kk in range(CT):
                    nc.sync.dma_start(
                        xT[kk], xT_dram[kk * P : (kk + 1) * P, :]
                    )
                    nc.sync.dma_start(ws[kk], w_dram[kk * P : (kk + 1) * P, :])
                for m in range(CT):
                    ps = pmm.tile([P, S], F32, tag="mm", name="ps")
                    for n in range(NH):
                        for kk in range(CT):
                            nc.tensor.matmul(
                                ps[:, n * 512 : (n + 1) * 512],
                                lhsT=ws[kk][:, m * P : (m + 1) * P],
                                rhs=xT[kk][:, n * 512 : (n + 1) * 512],
                                start=(kk == 0),
                                stop=(kk == CT - 1),
                            )
                    sg = stage.tile([P, S], FR, tag="stage", name="sg")
                    nc.scalar.activation(sg, ps, AF.Relu, bias=pcol(jb, m))
                    if out_tiles is not None:
                        nc.vector.tensor_scalar(
                            out_tiles[m], sg, pcol(js, m), pcol(jt, m),
                            op0=OP.mult, op1=OP.add,
                        )
                    else:
                        sg2 = stage.tile([P, S], FR, tag="stage", name="sg2")
                        nc.vector.tensor_scalar(
                            sg2, sg, pcol(js, m), pcol(jt, m),
                            op0=OP.mult, op1=OP.add,
                        )
                        nc.sync.dma_start(
                            out_dram[m * P : (m + 1) * P, :], sg2
                        )

            project(xq, wq, 0, 1, 2, None, qpT_sc)
            project(xk, wk, 3, 4, 5, None, kpT_sc)
            vpT = [
                ring.tile([P, S], FR, tag="ring", name=f"vpT{i}")
                for i in range(CT)
            ]
            project(xv, wv, 6, 7, 8, vpT, None)
            # vp_aug[st] head cols <- vpT[ct][:, st*128:].T (2 heads/ct)
            for ct in range(CT):
                for st in range(ST):
                    pt = ptr.tile([P, P], FR, tag="tr", name="ptv")
                    nc.tensor.transpose(
                        pt, vpT[ct][:, st * P : (st + 1) * P], ident
                    )
                    for hh in range(2):
                        h = 2 * ct + hh
                        nc.vector.tensor_copy(
                            vpa[st][:, h * HB : h * HB + DP],
                            pt[:, hh * DP : (hh + 1) * DP],
                        )

            # ---- phase B: attention per head ----
            for h in range(H):
                pair, r0 = h // 2, (h % 2) * DP
                if r0 == 0:
                    qs = ring.tile([P, S], FR, tag="ring", name=f"qs{pair}")
                    nc.sync.dma_start(
                        qs, qpT_sc[pair * P : (pair + 1) * P, :]
                    )
                    ks = ring.tile([P, S], FR, tag="ring", name=f"ks{pair}")
                    nc.sync.dma_start(
                        ks, kpT_sc[pair * P : (pair + 1) * P, :]
                    )
                qh = qs[r0 : r0 + DP, :]
                kh = ks[r0 : r0 + DP, :]

                e_tiles = []
                for kt in range(ST):
                    et = ring.tile([P, S], FR, tag="ring", name=f"e{h}_{kt}")
                    ps = pmm.tile([P, S], F32, tag="mm", name="psqk")
                    for n in range(NH):
                        nc.tensor.matmul(
                            ps[:, n * 512 : (n + 1) * 512],
                            lhsT=kh[:, kt * P : (kt + 1) * P],
                            rhs=qh[:, n * 512 : (n + 1) * 512],
                            start=True,
                            stop=True,
                        )
                    nc.scalar.activation(
                        et, ps, AF.Exp, bias=mcol[:, kt : kt + 1]
                    )
                    e_tiles.append(et)

                # ctxT' rows 0:64 + softmax sums in row 64 (ones column)
                pcs = []
                srow = rows.tile([1, S], FR, tag="srow", name=f"srow{h}")
                for n in range(NH):
                    pc = pctx.tile([P, 512], F32, tag="ctx", name="psctx")
                    for kt in range(ST):
                        nc.tensor.matmul(
                            pc[:HB, :],
                            lhsT=vpa[kt][:, h * HB : (h + 1) * HB],
                            rhs=e_tiles[kt][:, n * 512 : (n + 1) * 512],
                            start=(kt == 0),
                            stop=(kt == ST - 1),
                        )
                    # PSUM can't source a DMA; hop the row through SBUF
                    nc.scalar.activation(
                        srow[:, n * 512 : (n + 1) * 512], pc[DP:HB, :],
                        AF.Copy,
                    )
                    pcs.append(pc)
                nc.sync.dma_start(sum_sc[h : h + 1, :], srow)

                # reciprocal over [128, 8] (partition-parallel: ~50ns
                # instead of 6.5us for a 1024-wide row on one partition)
                rs8 = rows.tile([P, ST], FR, tag="rs8", name=f"rs8{h}")
                nc.sync.dma_start(
                    rs8, sum_sc[h].rearrange("(p t) -> p t", t=ST)
                )
                nc.vector.reciprocal(rs8, rs8)
                nc.sync.dma_start(
                    inv_sc[h].rearrange("(p t) -> p t", t=ST), rs8
                )
                ib = bcast.tile([P, S], FR, tag="ib", name=f"ib{h}")
                nc.sync.dma_start(
                    ib, inv_sc[h : h + 1, :].to_broadcast([P, S])
                )

                if r0 == 0:
                    cst = cstage.tile([P, S], FR, tag="cst", name=f"cst{pair}")
                for n in range(NH):
                    nc.vector.tensor_mul(
                        cst[r0 : r0 + DP, n * 512 : (n + 1) * 512],
                        pcs[n][:DP, :],
                        ib[:DP, n * 512 : (n + 1) * 512],
                    )
                if r0 == DP:
                    nc.sync.dma_start(
                        ctx_sc[pair * P : (pair + 1) * P, :], cst
                    )

                for kt in range(ST):
                    eng = nc.vector if kt % 2 == 0 else nc.gpsimd
                    eng.tensor_mul(e_tiles[kt], e_tiles[kt], ib)
                    nc.sync.dma_start(
                        attnT[h, kt * P : (kt + 1) * P, :], e_tiles[kt]
                    )

            # ---- phase C: output projection ----
            cs = [
                ring.tile([P, S], FR, tag="ring", name=f"cs{i}")
                for i in range(CT)
            ]
            wos = [
                ring.tile([P, D], FR, tag="ring", name=f"wos{i}")
                for i in range(CT)
            ]
            for kk in range(CT):
                nc.sync.dma_start(cs[kk], ctx_sc[kk * P : (kk + 1) * P, :])
                nc.sync.dma_start(wos[kk], wo[kk * P : (kk + 1) * P, :])
            for m in range(CT):
                po = pmm.tile([P, S], F32, tag="mm", name="pso")
                for n in range(NH):
                    for kk in range(CT):
                        nc.tensor.matmul(
                            po[:, n * 512 : (n + 1) * 512],
                            lhsT=wos[kk][:, m * P : (m + 1) * P],
                            rhs=cs[kk][:, n * 512 : (n + 1) * 512],
                            start=(kk == 0),
                            stop=(kk == CT - 1),
                        )
                ot = stage.tile([P, S], FR, tag="stage", name="ot")
                nc.scalar.activation(ot, po, AF.Relu, bias=pcol(9, m))
                nc.sync.dma_start(outT[m * P : (m + 1) * P, :], ot)
    nc.compile()
    return nc


_NC = None


def _get_nc():
    global _NC
    if _NC is None:
        _NC = build_nc()
    return _NC


def make_in_maps(q, k, v, mask, wq, bq, wk, bk, wv, bv, wo, bo,
                 g1, be1, mm1, mv1, g2, be2, mm2, mv2, g3, be3, mm3, mv3):
    f = np.float32
    r1 = (g1 / np.sqrt(mv1 + BN_EPS)).astype(f)
    r2 = (g2 / np.sqrt(mv2 + BN_EPS)).astype(f)
    r3 = (g3 / np.sqrt(mv3 + BN_EPS)).astype(f)
    scale = f(1.0 / np.sqrt(DP))
    vecs = [
        np.asarray(bq, f), r1 * scale, (be1 - mm1 * r1).astype(f) * scale,
        np.asarray(bk, f), r2, (be2 - mm2 * r2).astype(f),
        np.asarray(bv, f), r3, (be3 - mm3 * r3).astype(f),
        np.asarray(bo, f),
    ]
    # params[p, j*8+m] = vec_j[m*128+p]
    prm = np.stack([x.reshape(CT, P).T for x in vecs], 1).reshape(P, 10 * CT)
    prm = np.ascontiguousarray(prm, f)
    consts = np.concatenate([np.eye(P, dtype=f), np.ones((P, H), f)], 1)
    shared = {
        "consts": consts,
        "wq": np.ascontiguousarray(wq, f),
        "wk": np.ascontiguousarray(wk, f),
        "wv": np.ascontiguousarray(wv, f),
        "wo": np.ascontiguousarray(wo, f),
        "params": prm,
    }
    q, k, v = np.asarray(q, f), np.asarray(k, f), np.asarray(v, f)
    in_maps = []
    for b in range(NCORES):
        mb = (np.asarray(mask[b, 0, 0, :], f) * f(-1e9)).reshape(ST, P).T
        in_maps.append({
            "xq": np.ascontiguousarray(q[b].T),
            "xk": np.ascontiguousarray(k[b].T),
            "xv": np.ascontiguousarray(v[b].T),
            "msk": np.ascontiguousarray(mb, f),
            **shared,
        })
    return in_maps


def kernel(**inputs):
    nc = _get_nc()
    in_maps = make_in_maps(**inputs)
    res = run_bass_kernel_spmd(nc, in_maps, list(range(NCORES))).results
    out = np.stack([res[b]["outT"].T for b in range(NCORES)])
    attn = np.stack([res[b]["attnT"] for b in range(NCORES)])
    return out, attn.transpose(0, 1, 3, 2)


# revision 18
# speedup vs baseline: 1.0411x; 1.0411x over previous
"""Trainium2 Bass kernel for nn_MultiHeadAttention_73624329388477.

Reference computation (B=8, S=1024, D=1024, H=16, depth=64):
    qh = split_heads(BN1(relu(q @ wq + bq)))      (BN inference: affine)
    kh = split_heads(BN2(relu(k @ wk + bk)))
    vh = split_heads(BN3(relu(v @ wv + bv)))
    scores = qh @ kh^T / sqrt(64) + mask * -1e9
    attn = softmax(scores, axis=-1)
    ctx = attn @ vh  -> concat heads
    out = relu(ctx @ wo + bo)
    returns (out, attn)

Sharding: data-parallel over batch. Core b computes batch b entirely
(full projections + 16 heads + output projection). No collectives.

Per-core layout strategy (all matmuls in float32r: 1 cycle/row on PE):
  - q,k,v arrive HOST-pre-transposed as [channel, seq] so the in-channel
    contraction sits on the partition dim with no on-chip transposes.
  - Q/K/V projections produce qpT/kpT/vpT in [out_ch, seq] layout
    (lhsT = weight k-strip, rhs = xT). ReLU+bias fused on ScalarE
    (per-partition bias), BN affine fused on VectorE tensor_scalar
    (per-partition scale+shift). The 1/sqrt(depth) factor is folded
    into the q-path BN scale on the host.
  - attention per head computes scoresT [k, q] = kh @ qh^T. The mask is
    per-k = per-partition, so exp(scores + mask*-1e9) is ONE ScalarE
    activation straight out of PSUM with the mask column as bias.
  - V is PE-transposed once into vp_aug [seq, 16*(64+1)]: per head 64
    value columns plus a ones column. The ctx matmul
    matmul(lhsT=vp_aug_head[k, 65], rhs=e^T[k, q]) then yields ctxT'
    rows 0:64 AND the softmax denominators in row 64 for free.
  - the denominator row bounces through DRAM and broadcast-loads
    (0-step partition AP) to [128, q]; reciprocal runs full-width.
  - e^T is normalized in place (split across VectorE and GpSimd) and
    DMA'd out as attn^T (host returns a transposed view); ctxT' is
    normalized during PSUM eviction by the same inv tile.
  - output projection in [out_ch, seq] layout (lhsT=wo strip,
    rhs=ctxT) with per-partition bias + ReLU; host transposes the view.
  - qpT/kpT/ctxT round-trip through DRAM scratch to stay inside SBUF.
"""

import numpy as np

import concourse.bass as bass
import concourse.bacc as bacc
import concourse.tile as tile
from concourse import mybir
from concourse.bass_utils import run_bass_kernel_spmd

S = 1024
D = 1024
H = 16
DP = 64
P = 128
NCORES = 8
BN_EPS = 1e-3

FR = mybir.dt.float32r
F32 = mybir.dt.float32

ST = S // P      # 8 seq tiles
CT = D // P      # 8 channel tiles
NH = S // 512    # 2 free-dim halves of 512
HB = DP + 1      # head block in vp_aug: 64 value cols + ones col

AF = mybir.ActivationFunctionType
OP = mybir.AluOpType


def build_nc():
    # Bacc (not plain Bass): its compile() pass splits multi-semaphore
    # waits into chains — TPB instructions only take ONE sync wait each.
    nc = bacc.Bacc(None, target_bir_lowering=False)

    xq = nc.dram_tensor("xq", [D, S], FR, kind="ExternalInput")  # q[b].T
    xk = nc.dram_tensor("xk", [D, S], FR, kind="ExternalInput")  # k[b].T
    xv = nc.dram_tensor("xv", [D, S], FR, kind="ExternalInput")  # v[b].T
    wq = nc.dram_tensor("wq", [D, D], FR, kind="ExternalInput")
    wk = nc.dram_tensor("wk", [D, D], FR, kind="ExternalInput")
    wv = nc.dram_tensor("wv", [D, D], FR, kind="ExternalInput")
    wo = nc.dram_tensor("wo", [D, D], FR, kind="ExternalInput")
    # params[p, j*8+m] = vec_j[m*128+p]; j: bq,sq,tq,bk,sk,tk,bv,sv,tv,bo
    params = nc.dram_tensor("params", [P, 10 * CT], F32, kind="ExternalInput")
    # msk[p, t] = -1e9 * mask[t*128+p]
    msk = nc.dram_tensor("msk", [P, ST], F32, kind="ExternalInput")
    # [eye(128) | ones(128,16)] — walrus rejects memset of float32r
    # tiles, so the transpose identity and ones columns arrive via DMA.
    consts = nc.dram_tensor("consts", [P, P + H], FR, kind="ExternalInput")

    outT = nc.dram_tensor("outT", [D, S], FR, kind="ExternalOutput")
    attnT = nc.dram_tensor("attnT", [H, S, S], FR, kind="ExternalOutput")

    # DRAM scratch (SBUF can't hold qpT/kpT/ctxT for the whole kernel)
    qpT_sc = nc.dram_tensor("qpT_sc", [D, S], FR)
    kpT_sc = nc.dram_tensor("kpT_sc", [D, S], FR)
    ctx_sc = nc.dram_tensor("ctx_sc", [D, S], FR)
    inv_sc = nc.dram_tensor("inv_sc", [H, S], F32)

    with tile.TileContext(nc) as tc:
        with (
            # float32r tiles are bit-identical fp32 storage; only the PE
            # multiply path is reduced-precision, so f32r outputs from
            # DVE/ACT ops lose nothing.
            nc.allow_low_precision(reason="float32r storage is fp32"),
            tc.tile_pool(name="res", bufs=1) as res,
            tc.tile_pool(name="ring", bufs=26) as ring,
            tc.tile_pool(name="stage", bufs=4) as stage,
            tc.tile_pool(name="cstage", bufs=3) as cstage,
            tc.tile_pool(name="bcast", bufs=3) as bcast,
            tc.tile_pool(name="rows", bufs=2) as rows,
            tc.tile_pool(name="pmm", bufs=2, space="PSUM") as pmm,
            tc.tile_pool(name="pctx", bufs=2, space="PSUM") as pctx,
            tc.tile_pool(name="ptr", bufs=2, space="PSUM") as ptr,
        ):
            cst_in = res.tile([P, P + H], FR, name="cst_in")
            nc.sync.dma_start(cst_in, consts[:, :])
            ident = cst_in[:, :P]
            prm = res.tile([P, 10 * CT], F32, name="prm")
            nc.sync.dma_start(prm, params[:, :])
            mcol = res.tile([P, ST], F32, name="mcol")
            nc.sync.dma_start(mcol, msk[:, :])

            def pcol(j, m):  # [128,1] per-partition column of param j
                return prm[:, j * CT + m : j * CT + m + 1]

            # vp_aug[st][:, h*65 : h*65+64] = v-projection head h rows;
            # col h*65+64 = ones (yields softmax sums in the ctx matmul).
            vpa = [res.tile([P, H * HB], FR, name=f"vpa{i}") for i in range(ST)]
            # Let PE observe ident's DMA once so later transposes carry
            # only their own input's wait (one sync wait per TPB inst).
            pwarm = ptr.tile([P, P], FR, tag="tr", name="pwarm")
            nc.tensor.transpose(pwarm, ident, ident)
            for st in range(ST):
                # ones columns, written once via a strided DMA
                ones_dst = vpa[st].rearrange(
                    "p (h c) -> p h c", c=HB
                )[:, :, DP : DP + 1]
                nc.sync.dma_start(ones_dst, consts[:, P : P + H])

            # ---- phase A: projections ----
            def project(xT_dram, w_dram, jb, js, jt, out_tiles, out_dram):
                xT = [
                    ring.tile([P, S], FR, tag="ring", name=f"xT{i}")
                    for i in range(CT)
                ]
                ws = [
                    ring.tile([P, D], FR, tag="ring", name=f"w{i}")
                    for i in range(CT)
                ]
                for kk in range(CT):
                    nc.sync.dma_start(
                        xT[kk], xT_dram[kk * P : (kk + 1) * P, :]
                    )
                    nc.sync.dma_start(ws[kk], w_dram[kk * P : (kk + 1) * P, :])
                for m in range(CT):
                    ps = pmm.tile([P, S], F32, tag="mm", name="ps")
                    for n in range(NH):
                        for kk in range(CT):
                            nc.tensor.matmul(
                                ps[:, n * 512 : (n + 1) * 512],
                                lhsT=ws[kk][:, m * P : (m + 1) * P],
                                rhs=xT[kk][:, n * 512 : (n + 1) * 512],
                                start=(kk == 0),
                                stop=(kk == CT - 1),
                            )
                    sg = stage.tile([P, S], FR, tag="stage", name="sg")
                    nc.scalar.activation(sg, ps, AF.Relu, bias=pcol(jb, m))
                    if out_tiles is not None:
                        nc.vector.tensor_scalar(
                            out_tiles[m], sg, pcol(js, m), pcol(jt, m),
                            op0=OP.mult, op1=OP.add,
                        )
                    else:
                        sg2 = stage.tile([P, S], FR, tag="stage", name="sg2")
                        nc.vector.tensor_scalar(
                            sg2, sg, pcol(js, m), pcol(jt, m),
                            op0=OP.mult, op1=OP.add,
                        )
                        nc.sync.dma_start(
                            out_dram[m * P : (m + 1) * P, :], sg2
                        )

            project(xq, wq, 0, 1, 2, None, qpT_sc)
            project(xk, wk, 3, 4, 5, None, kpT_sc)
            vpT = [
                ring.tile([P, S], FR, tag="ring", name=f"vpT{i}")
                for i in range(CT)
            ]
            project(xv, wv, 6, 7, 8, vpT, None)
            # vp_aug[st] head cols <- vpT[ct][:, st*128:].T (2 heads/ct)
            for ct in range(CT):
                for st in range(ST):
                    pt = ptr.tile([P, P], FR, tag="tr", name="ptv")
                    nc.tensor.transpose(
                        pt, vpT[ct][:, st * P : (st + 1) * P], ident
                    )
                    for hh in range(2):
                        h = 2 * ct + hh
                        nc.vector.tensor_copy(
                            vpa[st][:, h * HB : h * HB + DP],
                            pt[:, hh * DP : (hh + 1) * DP],
                        )

            # ---- phase B: attention per head ----
            for h in range(H):
                pair, r0 = h // 2, (h % 2) * DP
                if r0 == 0:
                    qs = ring.tile([P, S], FR, tag="ring", name=f"qs{pair}")
                    nc.sync.dma_start(
                        qs, qpT_sc[pair * P : (pair + 1) * P, :]
                    )
                    ks = ring.tile([P, S], FR, tag="ring", name=f"ks{pair}")
                    nc.sync.dma_start(
                        ks, kpT_sc[pair * P : (pair + 1) * P, :]
                    )
                qh = qs[r0 : r0 + DP, :]
                kh = ks[r0 : r0 + DP, :]

                e_tiles = []
                for kt in range(ST):
                    et = ring.tile([P, S], FR, tag="ring", name=f"e{h}_{kt}")
                    ps = pmm.tile([P, S], F32, tag="mm", name="psqk")
                    for n in range(NH):
                        nc.tensor.matmul(
                            ps[:, n * 512 : (n + 1) * 512],
                            lhsT=kh[:, kt * P : (kt + 1) * P],
                            rhs=qh[:, n * 512 : (n + 1) * 512],
                            start=True,
                            stop=True,
                        )
                    nc.scalar.activation(
                        et, ps, AF.Exp, bias=mcol[:, kt : kt + 1]
                    )
                    e_tiles.append(et)

                # ctxT' rows 0:64 + softmax sums in row 64 (ones column)
                srow = rows.tile([1, S], F32, tag="srow", name=f"srow{h}")
                cu = cstage.tile([P, S], FR, tag="cu", name=f"cu{h}")
                for n in range(NH):
                    pc = pctx.tile([P, 512], F32, tag="ctx", name="psctx")
                    for kt in range(ST):
                        nc.tensor.matmul(
                            pc[:HB, :],
                            lhsT=vpa[kt][:, h * HB : (h + 1) * HB],
                            rhs=e_tiles[kt][:, n * 512 : (n + 1) * 512],
                            start=(kt == 0),
                            stop=(kt == ST - 1),
                        )
                    nc.scalar.activation(
                        srow[:, n * 512 : (n + 1) * 512], pc[DP:HB, :],
                        AF.Copy,
                    )
                    # evict unnormalized NOW so the PSUM slot isn't held
                    # hostage by the denominator chain
                    nc.scalar.activation(
                        cu[:DP, n * 512 : (n + 1) * 512], pc[:DP, :],
                        AF.Copy,
                    )

                inv = rows.tile([1, S], F32, tag="inv", name=f"inv{h}")
                nc.vector.reciprocal_approx_fast(inv, srow)
                nc.sync.dma_start(inv_sc[h : h + 1, :], inv)
                ib = bcast.tile([P, S], F32, tag="ib", name=f"ib{h}")
                nc.sync.dma_start(
                    ib, inv_sc[h : h + 1, :].to_broadcast([P, S])
                )

                if r0 == 0:
                    cst = cstage.tile([P, S], FR, tag="cst", name=f"cst{pair}")
                nc.vector.tensor_mul(
                    cst[r0 : r0 + DP, :], cu[:DP, :], ib[:DP, :]
                )
                if r0 == DP:
                    nc.sync.dma_start(
                        ctx_sc[pair * P : (pair + 1) * P, :], cst
                    )

                for kt in range(ST):
                    eng = nc.vector if kt % 2 == 0 else nc.gpsimd
                    eng.tensor_mul(e_tiles[kt], e_tiles[kt], ib)
                    nc.sync.dma_start(
                        attnT[h, kt * P : (kt + 1) * P, :], e_tiles[kt]
                    )

            # ---- phase C: output projection ----
            cs = [
                ring.tile([P, S], FR, tag="ring", name=f"cs{i}")
                for i in range(CT)
            ]
            wos = [
                ring.tile([P, D], FR, tag="ring", name=f"wos{i}")
                for i in range(CT)
            ]
            for kk in range(CT):
                nc.sync.dma_start(cs[kk], ctx_sc[kk * P : (kk + 1) * P, :])
                nc.sync.dma_start(wos[kk], wo[kk * P : (kk + 1) * P, :])
            for m in range(CT):
                po = pmm.tile([P, S], F32, tag="mm", name="pso")
                for n in range(NH):
                    for kk in range(CT):
                        nc.tensor.matmul(
                            po[:, n * 512 : (n + 1) * 512],
                            lhsT=wos[kk][:, m * P : (m + 1) * P],
                            rhs=cs[kk][:, n * 512 : (n + 1) * 512],
                            start=(kk == 0),
                            stop=(kk == CT - 1),
                        )
                ot = stage.tile([P, S], FR, tag="stage", name="ot")
                nc.scalar.activation(ot, po, AF.Relu, bias=pcol(9, m))
                nc.sync.dma_start(outT[m * P : (m + 1) * P, :], ot)
    nc.compile()
    return nc


_NC = None


def _get_nc():
    global _NC
    if _NC is None:
        _NC = build_nc()
    return _NC


def make_in_maps(q, k, v, mask, wq, bq, wk, bk, wv, bv, wo, bo,
                 g1, be1, mm1, mv1, g2, be2, mm2, mv2, g3, be3, mm3, mv3):
    f = np.float32
    r1 = (g1 / np.sqrt(mv1 + BN_EPS)).astype(f)
    r2 = (g2 / np.sqrt(mv2 + BN_EPS)).astype(f)
    r3 = (g3 / np.sqrt(mv3 + BN_EPS)).astype(f)
    scale = f(1.0 / np.sqrt(DP))
    vecs = [
        np.asarray(bq, f), r1 * scale, (be1 - mm1 * r1).astype(f) * scale,
        np.asarray(bk, f), r2, (be2 - mm2 * r2).astype(f),
        np.asarray(bv, f), r3, (be3 - mm3 * r3).astype(f),
        np.asarray(bo, f),
    ]
    # params[p, j*8+m] = vec_j[m*128+p]
    prm = np.stack([x.reshape(CT, P).T for x in vecs], 1).reshape(P, 10 * CT)
    prm = np.ascontiguousarray(prm, f)
    consts = np.concatenate([np.eye(P, dtype=f), np.ones((P, H), f)], 1)
    shared = {
        "consts": consts,
        "wq": np.ascontiguousarray(wq, f),
        "wk": np.ascontiguousarray(wk, f),
        "wv": np.ascontiguousarray(wv, f),
        "wo": np.ascontiguousarray(wo, f),
        "params": prm,
    }
    q, k, v = np.asarray(q, f), np.asarray(k, f), np.asarray(v, f)
    in_maps = []
    for b in range(NCORES):
        mb = (np.asarray(mask[b, 0, 0, :], f) * f(-1e9)).reshape(ST, P).T
        in_maps.append({
            "xq": np.ascontiguousarray(q[b].T),
            "xk": np.ascontiguousarray(k[b].T),
            "xv": np.ascontiguousarray(v[b].T),
            "msk": np.ascontiguousarray(mb, f),
            **shared,
        })
    return in_maps


def kernel(**inputs):
    nc = _get_nc()
    in_maps = make_in_maps(**inputs)
    res = run_bass_kernel_spmd(nc, in_maps, list(range(NCORES))).results
    out = np.stack([res[b]["outT"].T for b in range(NCORES)])
    attn = np.stack([res[b]["attnT"] for b in range(NCORES)])
    return out, attn.transpose(0, 1, 3, 2)


# revision 19
# speedup vs baseline: 1.1238x; 1.0794x over previous
"""Trainium2 Bass kernel for nn_MultiHeadAttention_73624329388477.

Reference computation (B=8, S=1024, D=1024, H=16, depth=64):
    qh = split_heads(BN1(relu(q @ wq + bq)))      (BN inference: affine)
    kh = split_heads(BN2(relu(k @ wk + bk)))
    vh = split_heads(BN3(relu(v @ wv + bv)))
    scores = qh @ kh^T / sqrt(64) + mask * -1e9
    attn = softmax(scores, axis=-1)
    ctx = attn @ vh  -> concat heads
    out = relu(ctx @ wo + bo)
    returns (out, attn)

Sharding: data-parallel over batch. Core b computes batch b entirely
(full projections + 16 heads + output projection). No collectives.

Per-core layout strategy (all matmuls in float32r: 1 cycle/row on PE):
  - q,k,v arrive HOST-pre-transposed as [channel, seq] so the in-channel
    contraction sits on the partition dim with no on-chip transposes.
  - Q/K/V projections produce qpT/kpT/vpT in [out_ch, seq] layout
    (lhsT = weight k-strip, rhs = xT). ReLU+bias fused on ScalarE
    (per-partition bias), BN affine fused on VectorE tensor_scalar
    (per-partition scale+shift). The 1/sqrt(depth) factor is folded
    into the q-path BN scale on the host.
  - attention per head computes scoresT [k, q] = kh @ qh^T. The mask is
    per-k = per-partition, so exp(scores + mask*-1e9) is ONE ScalarE
    activation straight out of PSUM with the mask column as bias.
  - V is PE-transposed once into vp_aug [seq, 16*(64+1)]: per head 64
    value columns plus a ones column. The ctx matmul
    matmul(lhsT=vp_aug_head[k, 65], rhs=e^T[k, q]) then yields ctxT'
    rows 0:64 AND the softmax denominators in row 64 for free.
  - the denominator row bounces through DRAM and broadcast-loads
    (0-step partition AP) to [128, q]; reciprocal runs full-width.
  - e^T is normalized in place (split across VectorE and GpSimd) and
    DMA'd out as attn^T (host returns a transposed view); ctxT' is
    normalized during PSUM eviction by the same inv tile.
  - output projection in [out_ch, seq] layout (lhsT=wo strip,
    rhs=ctxT) with per-partition bias + ReLU; host transposes the view.
  - qpT/kpT/ctxT round-trip through DRAM scratch to stay inside SBUF.
"""

import numpy as np

import concourse.bass as bass
import concourse.bacc as bacc
import concourse.tile as tile
from concourse import mybir
from concourse.bass_utils import run_bass_kernel_spmd

S = 1024
D = 1024
H = 16
DP = 64
P = 128
NCORES = 8
BN_EPS = 1e-3

FR = mybir.dt.float32r
F32 = mybir.dt.float32

ST = S // P      # 8 seq tiles
CT = D // P      # 8 channel tiles
NH = S // 512    # 2 free-dim halves of 512
HB = DP + 1      # head block in vp_aug: 64 value cols + ones col

AF = mybir.ActivationFunctionType
OP = mybir.AluOpType


def build_nc():
    # Bacc (not plain Bass): its compile() pass splits multi-semaphore
    # waits into chains — TPB instructions only take ONE sync wait each.
    nc = bacc.Bacc(None, target_bir_lowering=False)

    xq = nc.dram_tensor("xq", [D, S], FR, kind="ExternalInput")  # q[b].T
    xk = nc.dram_tensor("xk", [D, S], FR, kind="ExternalInput")  # k[b].T
    xv = nc.dram_tensor("xv", [D, S], FR, kind="ExternalInput")  # v[b].T
    wq = nc.dram_tensor("wq", [D, D], FR, kind="ExternalInput")
    wk = nc.dram_tensor("wk", [D, D], FR, kind="ExternalInput")
    wv = nc.dram_tensor("wv", [D, D], FR, kind="ExternalInput")
    wo = nc.dram_tensor("wo", [D, D], FR, kind="ExternalInput")
    # params[p, j*8+m] = vec_j[m*128+p]; j: bq,sq,tq,bk,sk,tk,bv,sv,tv,bo
    params = nc.dram_tensor("params", [P, 10 * CT], F32, kind="ExternalInput")
    # msk[p, t] = -1e9 * mask[t*128+p]
    msk = nc.dram_tensor("msk", [P, ST], F32, kind="ExternalInput")
    # [eye(128) | ones(128,16)] — walrus rejects memset of float32r
    # tiles, so the transpose identity and ones columns arrive via DMA.
    consts = nc.dram_tensor("consts", [P, P + H], FR, kind="ExternalInput")

    outT = nc.dram_tensor("outT", [D, S], FR, kind="ExternalOutput")
    attnT = nc.dram_tensor("attnT", [H, S, S], FR, kind="ExternalOutput")

    # DRAM scratch (SBUF can't hold qpT/kpT/ctxT for the whole kernel)
    qpT_sc = nc.dram_tensor("qpT_sc", [D, S], FR)
    kpT_sc = nc.dram_tensor("kpT_sc", [D, S], FR)
    ctx_sc = nc.dram_tensor("ctx_sc", [D, S], FR)
    inv_sc = nc.dram_tensor("inv_sc", [H, S], F32)

    with tile.TileContext(nc) as tc:
        with (
            # float32r tiles are bit-identical fp32 storage; only the PE
            # multiply path is reduced-precision, so f32r outputs from
            # DVE/ACT ops lose nothing.
            nc.allow_low_precision(reason="float32r storage is fp32"),
            tc.tile_pool(name="res", bufs=1) as res,
            tc.tile_pool(name="ring", bufs=26) as ring,
            tc.tile_pool(name="stage", bufs=4) as stage,
            tc.tile_pool(name="cstage", bufs=3) as cstage,
            tc.tile_pool(name="bcast", bufs=3) as bcast,
            tc.tile_pool(name="rows", bufs=2) as rows,
            tc.tile_pool(name="pmm", bufs=2, space="PSUM") as pmm,
            tc.tile_pool(name="pctx", bufs=2, space="PSUM") as pctx,
            tc.tile_pool(name="ptr", bufs=2, space="PSUM") as ptr,
        ):
            cst_in = res.tile([P, P + H], FR, name="cst_in")
            nc.sync.dma_start(cst_in, consts[:, :])
            ident = cst_in[:, :P]
            prm = res.tile([P, 10 * CT], F32, name="prm")
            nc.sync.dma_start(prm, params[:, :])
            mcol = res.tile([P, ST], F32, name="mcol")
            nc.sync.dma_start(mcol, msk[:, :])

            def pcol(j, m):  # [128,1] per-partition column of param j
                return prm[:, j * CT + m : j * CT + m + 1]

            # vp_aug[st][:, h*65 : h*65+64] = v-projection head h rows;
            # col h*65+64 = ones (yields softmax sums in the ctx matmul).
            vpa = [res.tile([P, H * HB], FR, name=f"vpa{i}") for i in range(ST)]
            # Let PE observe ident's DMA once so later transposes carry
            # only their own input's wait (one sync wait per TPB inst).
            pwarm = ptr.tile([P, P], FR, tag="tr", name="pwarm")
            nc.tensor.transpose(pwarm, ident, ident)
            for st in range(ST):
                # ones columns, written once via a strided DMA
                ones_dst = vpa[st].rearrange(
                    "p (h c) -> p h c", c=HB
                )[:, :, DP : DP + 1]
                nc.sync.dma_start(ones_dst, consts[:, P : P + H])

            # ---- phase A: projections ----
            def project(xT_dram, w_dram, jb, js, jt, out_tiles, out_dram):
                xT = [
                    ring.tile([P, S], FR, tag="ring", name=f"xT{i}")
                    for i in range(CT)
                ]
                ws = [
                    ring.tile([P, D], FR, tag="ring", name=f"w{i}")
                    for i in range(CT)
                ]
                for kk in range(CT):
                    nc.sync.dma_start(
                        xT[kk], xT_dram[kk * P : (kk + 1) * P, :]
                    )
                    nc.sync.dma_start(ws[kk], w_dram[kk * P : (kk + 1) * P, :])
                for m in range(CT):
                    ps = pmm.tile([P, S], F32, tag="mm", name="ps")
                    for n in range(NH):
                        for kk in range(CT):
                            nc.tensor.matmul(
                                ps[:, n * 512 : (n + 1) * 512],
                                lhsT=ws[kk][:, m * P : (m + 1) * P],
                                rhs=xT[kk][:, n * 512 : (n + 1) * 512],
                                start=(kk == 0),
                                stop=(kk == CT - 1),
                            )
                    sg = stage.tile([P, S], FR, tag="stage", name="sg")
                    nc.scalar.activation(sg, ps, AF.Relu, bias=pcol(jb, m))
                    if out_tiles is not None:
                        nc.vector.tensor_scalar(
                            out_tiles[m], sg, pcol(js, m), pcol(jt, m),
                            op0=OP.mult, op1=OP.add,
                        )
                    else:
                        sg2 = stage.tile([P, S], FR, tag="stage", name="sg2")
                        nc.vector.tensor_scalar(
                            sg2, sg, pcol(js, m), pcol(jt, m),
                            op0=OP.mult, op1=OP.add,
                        )
                        nc.sync.dma_start(
                            out_dram[m * P : (m + 1) * P, :], sg2
                        )

            project(xq, wq, 0, 1, 2, None, qpT_sc)
            project(xk, wk, 3, 4, 5, None, kpT_sc)
            vpT = [
                ring.tile([P, S], FR, tag="ring", name=f"vpT{i}")
                for i in range(CT)
            ]
            project(xv, wv, 6, 7, 8, vpT, None)
            # vp_aug[st] head cols <- vpT[ct][:, st*128:].T (2 heads/ct)
            for ct in range(CT):
                for st in range(ST):
                    pt = ptr.tile([P, P], FR, tag="tr", name="ptv")
                    nc.tensor.transpose(
                        pt, vpT[ct][:, st * P : (st + 1) * P], ident
                    )
                    for hh in range(2):
                        h = 2 * ct + hh
                        nc.vector.tensor_copy(
                            vpa[st][:, h * HB : h * HB + DP],
                            pt[:, hh * DP : (hh + 1) * DP],
                        )

            # ---- phase B: attention, software-pipelined one head deep.
            # Engines are in-order: emitting QK(h+1) before ctx(h) lets PE
            # run the next head's scores while ScalarE exps this head's,
            # instead of stalling on the exp->ctx dependency every head.
            strips = {}

            def qk_stage(h):
                pair, r0 = h // 2, (h % 2) * DP
                if r0 == 0:
                    qs = ring.tile([P, S], FR, tag="ring", name=f"qs{pair}")
                    nc.sync.dma_start(
                        qs, qpT_sc[pair * P : (pair + 1) * P, :]
                    )
                    ks = ring.tile([P, S], FR, tag="ring", name=f"ks{pair}")
                    nc.sync.dma_start(
                        ks, kpT_sc[pair * P : (pair + 1) * P, :]
                    )
                    strips[pair] = (qs, ks)
                qs, ks = strips[pair]
                qh = qs[r0 : r0 + DP, :]
                kh = ks[r0 : r0 + DP, :]
                e_tiles = []
                for kt in range(ST):
                    et = ring.tile([P, S], FR, tag="ring", name=f"e{h}_{kt}")
                    ps = pmm.tile([P, S], F32, tag="mm", name="psqk")
                    for n in range(NH):
                        nc.tensor.matmul(
                            ps[:, n * 512 : (n + 1) * 512],
                            lhsT=kh[:, kt * P : (kt + 1) * P],
                            rhs=qh[:, n * 512 : (n + 1) * 512],
                            start=True,
                            stop=True,
                        )
                    nc.scalar.activation(
                        et, ps, AF.Exp, bias=mcol[:, kt : kt + 1]
                    )
                    e_tiles.append(et)
                return e_tiles

            def ctx_stage(h, e_tiles):
                pair, r0 = h // 2, (h % 2) * DP
                # ctxT' rows 0:64 + softmax sums in row 64 (ones column)
                srow = rows.tile([1, S], F32, tag="srow", name=f"srow{h}")
                cu = cstage.tile([P, S], FR, tag="cu", name=f"cu{h}")
                for n in range(NH):
                    pc = pctx.tile([P, 512], F32, tag="ctx", name="psctx")
                    for kt in range(ST):
                        nc.tensor.matmul(
                            pc[:HB, :],
                            lhsT=vpa[kt][:, h * HB : (h + 1) * HB],
                            rhs=e_tiles[kt][:, n * 512 : (n + 1) * 512],
                            start=(kt == 0),
                            stop=(kt == ST - 1),
                        )
                    nc.scalar.activation(
                        srow[:, n * 512 : (n + 1) * 512], pc[DP:HB, :],
                        AF.Copy,
                    )
                    # evict unnormalized NOW so the PSUM slot isn't held
                    # hostage by the denominator chain
                    nc.scalar.activation(
                        cu[:DP, n * 512 : (n + 1) * 512], pc[:DP, :],
                        AF.Copy,
                    )

                inv = rows.tile([1, S], F32, tag="inv", name=f"inv{h}")
                nc.vector.reciprocal_approx_fast(inv, srow)
                nc.sync.dma_start(inv_sc[h : h + 1, :], inv)
                ib = bcast.tile([P, S], F32, tag="ib", name=f"ib{h}")
                nc.sync.dma_start(
                    ib, inv_sc[h : h + 1, :].to_broadcast([P, S])
                )

                if r0 == 0:
                    strips[f"cst{pair}"] = cstage.tile(
                        [P, S], FR, tag="cst", name=f"cst{pair}"
                    )
                cst = strips[f"cst{pair}"]
                nc.vector.tensor_mul(
                    cst[r0 : r0 + DP, :], cu[:DP, :], ib[:DP, :]
                )
                if r0 == DP:
                    nc.sync.dma_start(
                        ctx_sc[pair * P : (pair + 1) * P, :], cst
                    )

                for kt in range(ST):
                    eng = nc.vector if kt % 2 == 0 else nc.gpsimd
                    eng.tensor_mul(e_tiles[kt], e_tiles[kt], ib)
                    nc.sync.dma_start(
                        attnT[h, kt * P : (kt + 1) * P, :], e_tiles[kt]
                    )

            e_prev = None
            for h in range(H):
                e_h = qk_stage(h)
                if e_prev is not None:
                    ctx_stage(h - 1, e_prev)
                e_prev = e_h
            ctx_stage(H - 1, e_prev)

            # ---- phase C: output projection ----
            cs = [
                ring.tile([P, S], FR, tag="ring", name=f"cs{i}")
                for i in range(CT)
            ]
            wos = [
                ring.tile([P, D], FR, tag="ring", name=f"wos{i}")
                for i in range(CT)
            ]
            for kk in range(CT):
                nc.sync.dma_start(cs[kk], ctx_sc[kk * P : (kk + 1) * P, :])
                nc.sync.dma_start(wos[kk], wo[kk * P : (kk + 1) * P, :])
            for m in range(CT):
                po = pmm.tile([P, S], F32, tag="mm", name="pso")
                for n in range(NH):
                    for kk in range(CT):
                        nc.tensor.matmul(
                            po[:, n * 512 : (n + 1) * 512],
                            lhsT=wos[kk][:, m * P : (m + 1) * P],
                            rhs=cs[kk][:, n * 512 : (n + 1) * 512],
                            start=(kk == 0),
                            stop=(kk == CT - 1),
                        )
                ot = stage.tile([P, S], FR, tag="stage", name="ot")
                nc.scalar.activation(ot, po, AF.Relu, bias=pcol(9, m))
                nc.sync.dma_start(outT[m * P : (m + 1) * P, :], ot)
    nc.compile()
    return nc


_NC = None


def _get_nc():
    global _NC
    if _NC is None:
        _NC = build_nc()
    return _NC


def make_in_maps(q, k, v, mask, wq, bq, wk, bk, wv, bv, wo, bo,
                 g1, be1, mm1, mv1, g2, be2, mm2, mv2, g3, be3, mm3, mv3):
    f = np.float32
    r1 = (g1 / np.sqrt(mv1 + BN_EPS)).astype(f)
    r2 = (g2 / np.sqrt(mv2 + BN_EPS)).astype(f)
    r3 = (g3 / np.sqrt(mv3 + BN_EPS)).astype(f)
    scale = f(1.0 / np.sqrt(DP))
    vecs = [
        np.asarray(bq, f), r1 * scale, (be1 - mm1 * r1).astype(f) * scale,
        np.asarray(bk, f), r2, (be2 - mm2 * r2).astype(f),
        np.asarray(bv, f), r3, (be3 - mm3 * r3).astype(f),
        np.asarray(bo, f),
    ]
    # params[p, j*8+m] = vec_j[m*128+p]
    prm = np.stack([x.reshape(CT, P).T for x in vecs], 1).reshape(P, 10 * CT)
    prm = np.ascontiguousarray(prm, f)
    consts = np.concatenate([np.eye(P, dtype=f), np.ones((P, H), f)], 1)
    shared = {
        "consts": consts,
        "wq": np.ascontiguousarray(wq, f),
        "wk": np.ascontiguousarray(wk, f),
        "wv": np.ascontiguousarray(wv, f),
        "wo": np.ascontiguousarray(wo, f),
        "params": prm,
    }
    q, k, v = np.asarray(q, f), np.asarray(k, f), np.asarray(v, f)
    in_maps = []
    for b in range(NCORES):
        mb = (np.asarray(mask[b, 0, 0, :], f) * f(-1e9)).reshape(ST, P).T
        in_maps.append({
            "xq": np.ascontiguousarray(q[b].T),
            "xk": np.ascontiguousarray(k[b].T),
            "xv": np.ascontiguousarray(v[b].T),
            "msk": np.ascontiguousarray(mb, f),
            **shared,
        })
    return in_maps


def kernel(**inputs):
    nc = _get_nc()
    in_maps = make_in_maps(**inputs)
    res = run_bass_kernel_spmd(nc, in_maps, list(range(NCORES))).results
    out = np.stack([res[b]["outT"].T for b in range(NCORES)])
    attn = np.stack([res[b]["attnT"] for b in range(NCORES)])
    return out, attn.transpose(0, 1, 3, 2)


# revision 21
# speedup vs baseline: 1.1589x; 1.0313x over previous
"""Trainium2 Bass kernel for nn_MultiHeadAttention_73624329388477.

Reference computation (B=8, S=1024, D=1024, H=16, depth=64):
    qh = split_heads(BN1(relu(q @ wq + bq)))      (BN inference: affine)
    kh = split_heads(BN2(relu(k @ wk + bk)))
    vh = split_heads(BN3(relu(v @ wv + bv)))
    scores = qh @ kh^T / sqrt(64) + mask * -1e9
    attn = softmax(scores, axis=-1)
    ctx = attn @ vh  -> concat heads
    out = relu(ctx @ wo + bo)
    returns (out, attn)

Sharding: data-parallel over batch. Core b computes batch b entirely
(full projections + 16 heads + output projection). No collectives.

Per-core layout strategy (all matmuls in float32r: 1 cycle/row on PE):
  - q,k,v arrive HOST-pre-transposed as [channel, seq] so the in-channel
    contraction sits on the partition dim with no on-chip transposes.
  - Q/K/V projections produce qpT/kpT/vpT in [out_ch, seq] layout
    (lhsT = weight k-strip, rhs = xT). ReLU+bias fused on ScalarE
    (per-partition bias), BN affine fused on VectorE tensor_scalar
    (per-partition scale+shift). The 1/sqrt(depth) factor is folded
    into the q-path BN scale on the host.
  - attention per head computes scoresT [k, q] = kh @ qh^T. The mask is
    per-k = per-partition, so exp(scores + mask*-1e9) is ONE ScalarE
    activation straight out of PSUM with the mask column as bias.
  - V is PE-transposed once into vp_aug [seq, 16*(64+1)]: per head 64
    value columns plus a ones column. The ctx matmul
    matmul(lhsT=vp_aug_head[k, 65], rhs=e^T[k, q]) then yields ctxT'
    rows 0:64 AND the softmax denominators in row 64 for free.
  - the denominator row bounces through DRAM and broadcast-loads
    (0-step partition AP) to [128, q]; reciprocal runs full-width.
  - e^T is normalized in place (split across VectorE and GpSimd) and
    DMA'd out as attn^T (host returns a transposed view); ctxT' is
    normalized during PSUM eviction by the same inv tile.
  - output projection in [out_ch, seq] layout (lhsT=wo strip,
    rhs=ctxT) with per-partition bias + ReLU; host transposes the view.
  - qpT/kpT/ctxT round-trip through DRAM scratch to stay inside SBUF.
"""

import numpy as np

import concourse.bass as bass
import concourse.bacc as bacc
import concourse.tile as tile
from concourse import mybir
from concourse.bass_utils import run_bass_kernel_spmd

S = 1024
D = 1024
H = 16
DP = 64
P = 128
NCORES = 8
BN_EPS = 1e-3

FR = mybir.dt.float32r
F32 = mybir.dt.float32

ST = S // P      # 8 seq tiles
CT = D // P      # 8 channel tiles
NH = S // 512    # 2 free-dim halves of 512
HB = DP + 1      # head block in vp_aug: 64 value cols + ones col

AF = mybir.ActivationFunctionType
OP = mybir.AluOpType


def build_nc():
    # Bacc (not plain Bass): its compile() pass splits multi-semaphore
    # waits into chains — TPB instructions only take ONE sync wait each.
    nc = bacc.Bacc(None, target_bir_lowering=False)

    xq = nc.dram_tensor("xq", [D, S], FR, kind="ExternalInput")  # q[b].T
    xk = nc.dram_tensor("xk", [D, S], FR, kind="ExternalInput")  # k[b].T
    xv = nc.dram_tensor("xv", [D, S], FR, kind="ExternalInput")  # v[b].T
    wq = nc.dram_tensor("wq", [D, D], FR, kind="ExternalInput")
    wk = nc.dram_tensor("wk", [D, D], FR, kind="ExternalInput")
    wv = nc.dram_tensor("wv", [D, D], FR, kind="ExternalInput")
    wo = nc.dram_tensor("wo", [D, D], FR, kind="ExternalInput")
    # params[p, j*8+m] = vec_j[m*128+p]; j: bq,sq,tq,bk,sk,tk,bv,sv,tv,bo
    params = nc.dram_tensor("params", [P, 10 * CT], F32, kind="ExternalInput")
    # msk[p, t] = -1e9 * mask[t*128+p]
    msk = nc.dram_tensor("msk", [P, ST], F32, kind="ExternalInput")
    # [eye(128) | ones(128,16)] — walrus rejects memset of float32r
    # tiles, so the transpose identity and ones columns arrive via DMA.
    consts = nc.dram_tensor("consts", [P, P + H], FR, kind="ExternalInput")

    outT = nc.dram_tensor("outT", [D, S], FR, kind="ExternalOutput")
    attnT = nc.dram_tensor("attnT", [H, S, S], FR, kind="ExternalOutput")

    # DRAM scratch (SBUF can't hold qpT/kpT/ctxT for the whole kernel)
    qpT_sc = nc.dram_tensor("qpT_sc", [D, S], FR)
    kpT_sc = nc.dram_tensor("kpT_sc", [D, S], FR)
    ctx_sc = nc.dram_tensor("ctx_sc", [D, S], FR)
    inv_sc = nc.dram_tensor("inv_sc", [H, S], FR)

    with tile.TileContext(nc) as tc:
        with (
            # float32r tiles are bit-identical fp32 storage; only the PE
            # multiply path is reduced-precision, so f32r outputs from
            # DVE/ACT ops lose nothing.
            nc.allow_low_precision(reason="float32r storage is fp32"),
            tc.tile_pool(name="res", bufs=1) as res,
            tc.tile_pool(name="ring", bufs=22) as ring,
            tc.tile_pool(name="pstr", bufs=6) as pstr,
            tc.tile_pool(name="stage", bufs=4) as stage,
            tc.tile_pool(name="cstage", bufs=2) as cstage,
            tc.tile_pool(name="bcast", bufs=3) as bcast,
            tc.tile_pool(name="rows", bufs=2) as rows,
            tc.tile_pool(name="pmm", bufs=2, space="PSUM") as pmm,
            tc.tile_pool(name="pctx", bufs=2, space="PSUM") as pctx,
            tc.tile_pool(name="ptr", bufs=2, space="PSUM") as ptr,
        ):
            cst_in = res.tile([P, P + H], FR, name="cst_in")
            nc.sync.dma_start(cst_in, consts[:, :])
            ident = cst_in[:, :P]
            prm = res.tile([P, 10 * CT], F32, name="prm")
            nc.sync.dma_start(prm, params[:, :])
            mcol = res.tile([P, ST], F32, name="mcol")
            nc.sync.dma_start(mcol, msk[:, :])

            def pcol(j, m):  # [128,1] per-partition column of param j
                return prm[:, j * CT + m : j * CT + m + 1]

            # vp_aug[st][:, h*65 : h*65+64] = v-projection head h rows;
            # col h*65+64 = ones (yields softmax sums in the ctx matmul).
            vpa = [res.tile([P, H * HB], FR, name=f"vpa{i}") for i in range(ST)]
            # Let PE observe ident's DMA once so later transposes carry
            # only their own input's wait (one sync wait per TPB inst).
            pwarm = ptr.tile([P, P], FR, tag="tr", name="pwarm")
            nc.tensor.transpose(pwarm, ident, ident)
            for st in range(ST):
                # ones columns, written once via a strided DMA
                ones_dst = vpa[st].rearrange(
                    "p (h c) -> p h c", c=HB
                )[:, :, DP : DP + 1]
                nc.sync.dma_start(ones_dst, consts[:, P : P + H])

            # ---- phase A: projections ----
            def project(xT_dram, w_dram, jb, js, jt, out_tiles, out_dram):
                xT = [
                    ring.tile([P, S], FR, tag="ring", name=f"xT{i}")
                    for i in range(CT)
                ]
                ws = [
                    ring.tile([P, D], FR, tag="ring", name=f"w{i}")
                    for i in range(CT)
                ]
                for kk in range(CT):
                    nc.sync.dma_start(
                        xT[kk], xT_dram[kk * P : (kk + 1) * P, :]
                    )
                    nc.sync.dma_start(ws[kk], w_dram[kk * P : (kk + 1) * P, :])
                for m in range(CT):
                    ps = pmm.tile([P, S], F32, tag="mm", name="ps")
                    for n in range(NH):
                        for kk in range(CT):
                            nc.tensor.matmul(
                                ps[:, n * 512 : (n + 1) * 512],
                                lhsT=ws[kk][:, m * P : (m + 1) * P],
                                rhs=xT[kk][:, n * 512 : (n + 1) * 512],
                                start=(kk == 0),
                                stop=(kk == CT - 1),
                            )
                    sg = stage.tile([P, S], FR, tag="stage", name="sg")
                    nc.scalar.activation(sg, ps, AF.Relu, bias=pcol(jb, m))
                    if out_tiles is not None:
                        nc.vector.tensor_scalar(
                            out_tiles[m], sg, pcol(js, m), pcol(jt, m),
                            op0=OP.mult, op1=OP.add,
                        )
                    else:
                        sg2 = stage.tile([P, S], FR, tag="stage", name="sg2")
                        nc.vector.tensor_scalar(
                            sg2, sg, pcol(js, m), pcol(jt, m),
                            op0=OP.mult, op1=OP.add,
                        )
                        nc.sync.dma_start(
                            out_dram[m * P : (m + 1) * P, :], sg2
                        )

            project(xq, wq, 0, 1, 2, None, qpT_sc)
            project(xk, wk, 3, 4, 5, None, kpT_sc)
            vpT = [
                ring.tile([P, S], FR, tag="ring", name=f"vpT{i}")
                for i in range(CT)
            ]
            project(xv, wv, 6, 7, 8, vpT, None)
            # vp_aug[st] head cols <- vpT[ct][:, st*128:].T (2 heads/ct)
            for ct in range(CT):
                for st in range(ST):
                    pt = ptr.tile([P, P], FR, tag="tr", name="ptv")
                    nc.tensor.transpose(
                        pt, vpT[ct][:, st * P : (st + 1) * P], ident
                    )
                    for hh in range(2):
                        h = 2 * ct + hh
                        nc.vector.tensor_copy(
                            vpa[st][:, h * HB : h * HB + DP],
                            pt[:, hh * DP : (hh + 1) * DP],
                        )

            # ---- phase B: attention, software-pipelined one head deep.
            # Engines are in-order: emitting QK(h+1) before ctx(h) lets PE
            # run the next head's scores while ScalarE exps this head's,
            # instead of stalling on the exp->ctx dependency every head.
            strips = {}

            def load_strips(pair):
                if pair >= H // 2 or pair in strips:
                    return
                qs = pstr.tile([P, S], FR, tag="strips", name=f"qs{pair}")
                nc.sync.dma_start(qs, qpT_sc[pair * P : (pair + 1) * P, :])
                ks = pstr.tile([P, S], FR, tag="strips", name=f"ks{pair}")
                nc.sync.dma_start(ks, kpT_sc[pair * P : (pair + 1) * P, :])
                strips[pair] = (qs, ks)

            def qk_stage(h):
                pair, r0 = h // 2, (h % 2) * DP
                load_strips(pair)
                load_strips(pair + 1)
                qs, ks = strips[pair]
                qh = qs[r0 : r0 + DP, :]
                kh = ks[r0 : r0 + DP, :]
                e_tiles = []
                for kt in range(ST):
                    et = ring.tile([P, S], FR, tag="ring", name=f"e{h}_{kt}")
                    ps = pmm.tile([P, S], F32, tag="mm", name="psqk")
                    for n in range(NH):
                        nc.tensor.matmul(
                            ps[:, n * 512 : (n + 1) * 512],
                            lhsT=kh[:, kt * P : (kt + 1) * P],
                            rhs=qh[:, n * 512 : (n + 1) * 512],
                            start=True,
                            stop=True,
                        )
                    nc.scalar.activation(
                        et, ps, AF.Exp, bias=mcol[:, kt : kt + 1]
                    )
                    e_tiles.append(et)
                return e_tiles

            def ctx_stage(h, e_tiles):
                pair, r0 = h // 2, (h % 2) * DP
                # ctxT' rows 0:64 + softmax sums in row 64 (ones column)
                srow = rows.tile([1, S], FR, tag="srow", name=f"srow{h}")
                cu = cstage.tile([P, S], FR, tag="cu", name=f"cu{h}")
                for n in range(NH):
                    pc = pctx.tile([P, 512], F32, tag="ctx", name="psctx")
                    for kt in range(ST):
                        nc.tensor.matmul(
                            pc[:HB, :],
                            lhsT=vpa[kt][:, h * HB : (h + 1) * HB],
                            rhs=e_tiles[kt][:, n * 512 : (n + 1) * 512],
                            start=(kt == 0),
                            stop=(kt == ST - 1),
                        )
                    nc.scalar.activation(
                        srow[:, n * 512 : (n + 1) * 512], pc[DP:HB, :],
                        AF.Copy,
                    )
                    # evict unnormalized NOW so the PSUM slot isn't held
                    # hostage by the denominator chain
                    nc.scalar.activation(
                        cu[:DP, n * 512 : (n + 1) * 512], pc[:DP, :],
                        AF.Copy,
                    )

                inv = rows.tile([1, S], FR, tag="inv", name=f"inv{h}")
                # reciprocal_approx_fast body, minus its fp32-tag assert —
                # float32r has the same bit layout the seed trick needs
                from concourse.dve_ops import (
                    RECIP_APPROX_FAST_CONSTS as _RC,
                    RECIPROCAL_APPROX_FAST as _RF,
                )
                nc.vector._custom_dve(
                    _RF, out=inv, in0=srow,
                    s0=_RC["s0"], s1=_RC["s1"], imm2=_RC["imm2"],
                )
                nc.sync.dma_start(inv_sc[h : h + 1, :], inv)
                ib = bcast.tile([P, S], FR, tag="ib", name=f"ib{h}")
                nc.sync.dma_start(
                    ib, inv_sc[h : h + 1, :].to_broadcast([P, S])
                )

                if r0 == 0:
                    strips[f"cst{pair}"] = cstage.tile(
                        [P, S], FR, tag="cst", name=f"cst{pair}"
                    )
                cst = strips[f"cst{pair}"]
                nc.vector.tensor_mul(
                    cst[r0 : r0 + DP, :], cu[:DP, :], ib[:DP, :]
                )
                if r0 == DP:
                    nc.sync.dma_start(
                        ctx_sc[pair * P : (pair + 1) * P, :], cst
                    )

                for kt in range(ST):
                    eng = nc.vector if kt % 2 == 0 else nc.gpsimd
                    eng.tensor_mul(e_tiles[kt], e_tiles[kt], ib)
                    nc.sync.dma_start(
                        attnT[h, kt * P : (kt + 1) * P, :], e_tiles[kt]
                    )

            e_prev = None
            for h in range(H):
                e_h = qk_stage(h)
                if e_prev is not None:
                    ctx_stage(h - 1, e_prev)
                e_prev = e_h
            ctx_stage(H - 1, e_prev)

            # ---- phase C: output projection ----
            cs = [
                ring.tile([P, S], FR, tag="ring", name=f"cs{i}")
                for i in range(CT)
            ]
            wos = [
                ring.tile([P, D], FR, tag="ring", name=f"wos{i}")
                for i in range(CT)
            ]
            for kk in range(CT):
                nc.sync.dma_start(cs[kk], ctx_sc[kk * P : (kk + 1) * P, :])
                nc.sync.dma_start(wos[kk], wo[kk * P : (kk + 1) * P, :])
            for m in range(CT):
                po = pmm.tile([P, S], F32, tag="mm", name="pso")
                for n in range(NH):
                    for kk in range(CT):
                        nc.tensor.matmul(
                            po[:, n * 512 : (n + 1) * 512],
                            lhsT=wos[kk][:, m * P : (m + 1) * P],
                            rhs=cs[kk][:, n * 512 : (n + 1) * 512],
                            start=(kk == 0),
                            stop=(kk == CT - 1),
                        )
                ot = stage.tile([P, S], FR, tag="stage", name="ot")
                nc.scalar.activation(ot, po, AF.Relu, bias=pcol(9, m))
                nc.sync.dma_start(outT[m * P : (m + 1) * P, :], ot)
    nc.compile()
    return nc


_NC = None


def _get_nc():
    global _NC
    if _NC is None:
        _NC = build_nc()
    return _NC


def make_in_maps(q, k, v, mask, wq, bq, wk, bk, wv, bv, wo, bo,
                 g1, be1, mm1, mv1, g2, be2, mm2, mv2, g3, be3, mm3, mv3):
    f = np.float32
    r1 = (g1 / np.sqrt(mv1 + BN_EPS)).astype(f)
    r2 = (g2 / np.sqrt(mv2 + BN_EPS)).astype(f)
    r3 = (g3 / np.sqrt(mv3 + BN_EPS)).astype(f)
    scale = f(1.0 / np.sqrt(DP))
    vecs = [
        np.asarray(bq, f), r1 * scale, (be1 - mm1 * r1).astype(f) * scale,
        np.asarray(bk, f), r2, (be2 - mm2 * r2).astype(f),
        np.asarray(bv, f), r3, (be3 - mm3 * r3).astype(f),
        np.asarray(bo, f),
    ]
    # params[p, j*8+m] = vec_j[m*128+p]
    prm = np.stack([x.reshape(CT, P).T for x in vecs], 1).reshape(P, 10 * CT)
    prm = np.ascontiguousarray(prm, f)
    consts = np.concatenate([np.eye(P, dtype=f), np.ones((P, H), f)], 1)
    shared = {
        "consts": consts,
        "wq": np.ascontiguousarray(wq, f),
        "wk": np.ascontiguousarray(wk, f),
        "wv": np.ascontiguousarray(wv, f),
        "wo": np.ascontiguousarray(wo, f),
        "params": prm,
    }
    q, k, v = np.asarray(q, f), np.asarray(k, f), np.asarray(v, f)
    in_maps = []
    for b in range(NCORES):
        mb = (np.asarray(mask[b, 0, 0, :], f) * f(-1e9)).reshape(ST, P).T
        in_maps.append({
            "xq": np.ascontiguousarray(q[b].T),
            "xk": np.ascontiguousarray(k[b].T),
            "xv": np.ascontiguousarray(v[b].T),
            "msk": np.ascontiguousarray(mb, f),
            **shared,
        })
    return in_maps


def kernel(**inputs):
    nc = _get_nc()
    in_maps = make_in_maps(**inputs)
    res = run_bass_kernel_spmd(nc, in_maps, list(range(NCORES))).results
    out = np.stack([res[b]["outT"].T for b in range(NCORES)])
    attn = np.stack([res[b]["attnT"] for b in range(NCORES)])
    return out, attn.transpose(0, 1, 3, 2)


# revision 22
# speedup vs baseline: 1.2087x; 1.0430x over previous
"""Trainium2 Bass kernel for nn_MultiHeadAttention_73624329388477.

Reference computation (B=8, S=1024, D=1024, H=16, depth=64):
    qh = split_heads(BN1(relu(q @ wq + bq)))      (BN inference: affine)
    kh = split_heads(BN2(relu(k @ wk + bk)))
    vh = split_heads(BN3(relu(v @ wv + bv)))
    scores = qh @ kh^T / sqrt(64) + mask * -1e9
    attn = softmax(scores, axis=-1)
    ctx = attn @ vh  -> concat heads
    out = relu(ctx @ wo + bo)
    returns (out, attn)

Sharding: data-parallel over batch. Core b computes batch b entirely
(full projections + 16 heads + output projection). No collectives.

Per-core layout strategy (all matmuls in float32r: 1 cycle/row on PE):
  - q,k,v arrive HOST-pre-transposed as [channel, seq] so the in-channel
    contraction sits on the partition dim with no on-chip transposes.
  - Q/K/V projections produce qpT/kpT/vpT in [out_ch, seq] layout
    (lhsT = weight k-strip, rhs = xT). ReLU+bias fused on ScalarE
    (per-partition bias), BN affine fused on VectorE tensor_scalar
    (per-partition scale+shift). The 1/sqrt(depth) factor is folded
    into the q-path BN scale on the host.
  - attention per head computes scoresT [k, q] = kh @ qh^T. The mask is
    per-k = per-partition, so exp(scores + mask*-1e9) is ONE ScalarE
    activation straight out of PSUM with the mask column as bias.
  - V is PE-transposed once into vp_aug [seq, 16*(64+1)]: per head 64
    value columns plus a ones column. The ctx matmul
    matmul(lhsT=vp_aug_head[k, 65], rhs=e^T[k, q]) then yields ctxT'
    rows 0:64 AND the softmax denominators in row 64 for free.
  - the denominator row bounces through DRAM and broadcast-loads
    (0-step partition AP) to [128, q]; reciprocal runs full-width.
  - e^T is normalized in place (split across VectorE and GpSimd) and
    DMA'd out as attn^T (host returns a transposed view); ctxT' is
    normalized during PSUM eviction by the same inv tile.
  - output projection in [out_ch, seq] layout (lhsT=wo strip,
    rhs=ctxT) with per-partition bias + ReLU; host transposes the view.
  - qpT/kpT/ctxT round-trip through DRAM scratch to stay inside SBUF.
"""

import numpy as np

import concourse.bass as bass
import concourse.bacc as bacc
import concourse.tile as tile
from concourse import mybir
from concourse.bass_utils import run_bass_kernel_spmd

S = 1024
D = 1024
H = 16
DP = 64
P = 128
NCORES = 8
BN_EPS = 1e-3

FR = mybir.dt.float32r
BF = mybir.dt.bfloat16
F32 = mybir.dt.float32

ST = S // P      # 8 seq tiles
CT = D // P      # 8 channel tiles
NH = S // 512    # 2 free-dim halves of 512
HB = DP + 1      # head block in vp_aug: 64 value cols + ones col

AF = mybir.ActivationFunctionType
OP = mybir.AluOpType


def build_nc():
    # Bacc (not plain Bass): its compile() pass splits multi-semaphore
    # waits into chains — TPB instructions only take ONE sync wait each.
    nc = bacc.Bacc(None, target_bir_lowering=False)

    xq = nc.dram_tensor("xq", [D, S], FR, kind="ExternalInput")  # q[b].T
    xk = nc.dram_tensor("xk", [D, S], FR, kind="ExternalInput")  # k[b].T
    xv = nc.dram_tensor("xv", [D, S], FR, kind="ExternalInput")  # v[b].T
    wq = nc.dram_tensor("wq", [D, D], FR, kind="ExternalInput")
    wk = nc.dram_tensor("wk", [D, D], FR, kind="ExternalInput")
    wv = nc.dram_tensor("wv", [D, D], FR, kind="ExternalInput")
    wo = nc.dram_tensor("wo", [D, D], FR, kind="ExternalInput")
    # params[p, j*8+m] = vec_j[m*128+p]; j: bq,sq,tq,bk,sk,tk,bv,sv,tv,bo
    params = nc.dram_tensor("params", [P, 10 * CT], F32, kind="ExternalInput")
    # msk[p, t] = -1e9 * mask[t*128+p]
    msk = nc.dram_tensor("msk", [P, ST], F32, kind="ExternalInput")
    # [eye(128) | ones(128,16)] — walrus rejects memset of float32r
    # tiles, so the transpose identity and ones columns arrive via DMA.
    consts = nc.dram_tensor("consts", [P, P + H], FR, kind="ExternalInput")
    ones_bf = nc.dram_tensor("ones_bf", [P, H], BF, kind="ExternalInput")

    outT = nc.dram_tensor("outT", [D, S], FR, kind="ExternalOutput")
    attnT = nc.dram_tensor("attnT", [H, S, S], BF, kind="ExternalOutput")

    # DRAM scratch (SBUF can't hold qpT/kpT/ctxT for the whole kernel)
    qpT_sc = nc.dram_tensor("qpT_sc", [D, S], FR)
    kpT_sc = nc.dram_tensor("kpT_sc", [D, S], FR)
    ctx_sc = nc.dram_tensor("ctx_sc", [D, S], FR)
    inv_sc = nc.dram_tensor("inv_sc", [H, S], FR)

    with tile.TileContext(nc) as tc:
        with (
            # float32r tiles are bit-identical fp32 storage; only the PE
            # multiply path is reduced-precision, so f32r outputs from
            # DVE/ACT ops lose nothing.
            nc.allow_low_precision(reason="float32r storage is fp32"),
            tc.tile_pool(name="res", bufs=1) as res,
            tc.tile_pool(name="ring", bufs=22) as ring,
            tc.tile_pool(name="pstr", bufs=6) as pstr,
            tc.tile_pool(name="stage", bufs=4) as stage,
            tc.tile_pool(name="cstage", bufs=2) as cstage,
            tc.tile_pool(name="bcast", bufs=3) as bcast,
            tc.tile_pool(name="rows", bufs=2) as rows,
            tc.tile_pool(name="pmm", bufs=2, space="PSUM") as pmm,
            tc.tile_pool(name="pctx", bufs=2, space="PSUM") as pctx,
            tc.tile_pool(name="ptr", bufs=2, space="PSUM") as ptr,
        ):
            cst_in = res.tile([P, P + H], FR, name="cst_in")
            nc.sync.dma_start(cst_in, consts[:, :])
            ident = cst_in[:, :P]
            prm = res.tile([P, 10 * CT], F32, name="prm")
            nc.sync.dma_start(prm, params[:, :])
            mcol = res.tile([P, ST], F32, name="mcol")
            nc.sync.dma_start(mcol, msk[:, :])

            def pcol(j, m):  # [128,1] per-partition column of param j
                return prm[:, j * CT + m : j * CT + m + 1]

            # vp_aug[st][:, h*65 : h*65+64] = v-projection head h rows;
            # col h*65+64 = ones (yields softmax sums in the ctx matmul).
            vpa = [res.tile([P, H * HB], BF, name=f"vpa{i}") for i in range(ST)]
            # Let PE observe ident's DMA once so later transposes carry
            # only their own input's wait (one sync wait per TPB inst).
            pwarm = ptr.tile([P, P], FR, tag="tr", name="pwarm")
            nc.tensor.transpose(pwarm, ident, ident)
            for st in range(ST):
                # ones columns, written once via a strided DMA
                ones_dst = vpa[st].rearrange(
                    "p (h c) -> p h c", c=HB
                )[:, :, DP : DP + 1]
                nc.sync.dma_start(ones_dst, ones_bf[:, :])

            # ---- phase A: projections ----
            def project(xT_dram, w_dram, jb, js, jt, out_tiles, out_dram):
                xT = [
                    ring.tile([P, S], FR, tag="ring", name=f"xT{i}")
                    for i in range(CT)
                ]
                ws = [
                    ring.tile([P, D], FR, tag="ring", name=f"w{i}")
                    for i in range(CT)
                ]
                for kk in range(CT):
                    nc.sync.dma_start(
                        xT[kk], xT_dram[kk * P : (kk + 1) * P, :]
                    )
                    nc.sync.dma_start(ws[kk], w_dram[kk * P : (kk + 1) * P, :])
                for m in range(CT):
                    ps = pmm.tile([P, S], F32, tag="mm", name="ps")
                    for n in range(NH):
                        for kk in range(CT):
                            nc.tensor.matmul(
                                ps[:, n * 512 : (n + 1) * 512],
                                lhsT=ws[kk][:, m * P : (m + 1) * P],
                                rhs=xT[kk][:, n * 512 : (n + 1) * 512],
                                start=(kk == 0),
                                stop=(kk == CT - 1),
                            )
                    sg = stage.tile([P, S], FR, tag="stage", name="sg")
                    nc.scalar.activation(sg, ps, AF.Relu, bias=pcol(jb, m))
                    if out_tiles is not None:
                        nc.vector.tensor_scalar(
                            out_tiles[m], sg, pcol(js, m), pcol(jt, m),
                            op0=OP.mult, op1=OP.add,
                        )
                    else:
                        sg2 = stage.tile([P, S], FR, tag="stage", name="sg2")
                        nc.vector.tensor_scalar(
                            sg2, sg, pcol(js, m), pcol(jt, m),
                            op0=OP.mult, op1=OP.add,
                        )
                        nc.sync.dma_start(
                            out_dram[m * P : (m + 1) * P, :], sg2
                        )

            project(xq, wq, 0, 1, 2, None, qpT_sc)
            project(xk, wk, 3, 4, 5, None, kpT_sc)
            vpT = [
                ring.tile([P, S], FR, tag="ring", name=f"vpT{i}")
                for i in range(CT)
            ]
            project(xv, wv, 6, 7, 8, vpT, None)
            # vp_aug[st] head cols <- vpT[ct][:, st*128:].T (2 heads/ct)
            for ct in range(CT):
                for st in range(ST):
                    pt = ptr.tile([P, P], FR, tag="tr", name="ptv")
                    nc.tensor.transpose(
                        pt, vpT[ct][:, st * P : (st + 1) * P], ident
                    )
                    for hh in range(2):
                        h = 2 * ct + hh
                        nc.vector.tensor_copy(
                            vpa[st][:, h * HB : h * HB + DP],
                            pt[:, hh * DP : (hh + 1) * DP],
                        )

            # ---- phase B: attention, software-pipelined one head deep.
            # Engines are in-order: emitting QK(h+1) before ctx(h) lets PE
            # run the next head's scores while ScalarE exps this head's,
            # instead of stalling on the exp->ctx dependency every head.
            strips = {}

            def load_strips(pair):
                if pair >= H // 2 or pair in strips:
                    return
                qs = pstr.tile([P, S], FR, tag="strips", name=f"qs{pair}")
                nc.sync.dma_start(qs, qpT_sc[pair * P : (pair + 1) * P, :])
                ks = pstr.tile([P, S], FR, tag="strips", name=f"ks{pair}")
                nc.sync.dma_start(ks, kpT_sc[pair * P : (pair + 1) * P, :])
                strips[pair] = (qs, ks)

            def qk_stage(h):
                pair, r0 = h // 2, (h % 2) * DP
                load_strips(pair)
                load_strips(pair + 1)
                qs, ks = strips[pair]
                qh = qs[r0 : r0 + DP, :]
                kh = ks[r0 : r0 + DP, :]
                e_tiles = []
                for kt in range(ST):
                    et = ring.tile([P, S], BF, tag="ring", name=f"e{h}_{kt}")
                    ps = pmm.tile([P, S], F32, tag="mm", name="psqk")
                    for n in range(NH):
                        nc.tensor.matmul(
                            ps[:, n * 512 : (n + 1) * 512],
                            lhsT=kh[:, kt * P : (kt + 1) * P],
                            rhs=qh[:, n * 512 : (n + 1) * 512],
                            start=True,
                            stop=True,
                        )
                    nc.scalar.activation(
                        et, ps, AF.Exp, bias=mcol[:, kt : kt + 1]
                    )
                    e_tiles.append(et)
                return e_tiles

            def ctx_stage(h, e_tiles):
                pair, r0 = h // 2, (h % 2) * DP
                # ctxT' rows 0:64 + softmax sums in row 64 (ones column)
                srow = rows.tile([1, S], FR, tag="srow", name=f"srow{h}")
                cu = cstage.tile([P, S], FR, tag="cu", name=f"cu{h}")
                for n in range(NH):
                    pc = pctx.tile([P, 512], F32, tag="ctx", name="psctx")
                    for kt in range(ST):
                        nc.tensor.matmul(
                            pc[:HB, :],
                            lhsT=vpa[kt][:, h * HB : (h + 1) * HB],
                            rhs=e_tiles[kt][:, n * 512 : (n + 1) * 512],
                            start=(kt == 0),
                            stop=(kt == ST - 1),
                        )
                    nc.scalar.activation(
                        srow[:, n * 512 : (n + 1) * 512], pc[DP:HB, :],
                        AF.Copy,
                    )
                    # evict unnormalized NOW so the PSUM slot isn't held
                    # hostage by the denominator chain
                    nc.scalar.activation(
                        cu[:DP, n * 512 : (n + 1) * 512], pc[:DP, :],
                        AF.Copy,
                    )

                inv = rows.tile([1, S], FR, tag="inv", name=f"inv{h}")
                # reciprocal_approx_fast body, minus its fp32-tag assert —
                # float32r has the same bit layout the seed trick needs
                from concourse.dve_ops import (
                    RECIP_APPROX_FAST_CONSTS as _RC,
                    RECIPROCAL_APPROX_FAST as _RF,
                )
                nc.vector._custom_dve(
                    _RF, out=inv, in0=srow,
                    s0=_RC["s0"], s1=_RC["s1"], imm2=_RC["imm2"],
                )
                nc.sync.dma_start(inv_sc[h : h + 1, :], inv)
                ib = bcast.tile([P, S], FR, tag="ib", name=f"ib{h}")
                nc.sync.dma_start(
                    ib, inv_sc[h : h + 1, :].to_broadcast([P, S])
                )
                ibf = bcast.tile([P, S], BF, tag="ibf", name=f"ibf{h}")
                nc.vector.tensor_copy(ibf, ib)

                if r0 == 0:
                    strips[f"cst{pair}"] = cstage.tile(
                        [P, S], FR, tag="cst", name=f"cst{pair}"
                    )
                cst = strips[f"cst{pair}"]
                nc.vector.tensor_mul(
                    cst[r0 : r0 + DP, :], cu[:DP, :], ib[:DP, :]
                )
                if r0 == DP:
                    nc.sync.dma_start(
                        ctx_sc[pair * P : (pair + 1) * P, :], cst
                    )

                for kt in range(ST):
                    eng = nc.vector if kt % 2 == 0 else nc.gpsimd
                    eng.tensor_mul(e_tiles[kt], e_tiles[kt], ibf)
                    nc.sync.dma_start(
                        attnT[h, kt * P : (kt + 1) * P, :], e_tiles[kt]
                    )

            e_prev = None
            for h in range(H):
                e_h = qk_stage(h)
                if e_prev is not None:
                    ctx_stage(h - 1, e_prev)
                e_prev = e_h
            ctx_stage(H - 1, e_prev)

            # ---- phase C: output projection ----
            cs = [
                ring.tile([P, S], FR, tag="ring", name=f"cs{i}")
                for i in range(CT)
            ]
            wos = [
                ring.tile([P, D], FR, tag="ring", name=f"wos{i}")
                for i in range(CT)
            ]
            for kk in range(CT):
                nc.sync.dma_start(cs[kk], ctx_sc[kk * P : (kk + 1) * P, :])
                nc.sync.dma_start(wos[kk], wo[kk * P : (kk + 1) * P, :])
            for m in range(CT):
                po = pmm.tile([P, S], F32, tag="mm", name="pso")
                for n in range(NH):
                    for kk in range(CT):
                        nc.tensor.matmul(
                            po[:, n * 512 : (n + 1) * 512],
                            lhsT=wos[kk][:, m * P : (m + 1) * P],
                            rhs=cs[kk][:, n * 512 : (n + 1) * 512],
                            start=(kk == 0),
                            stop=(kk == CT - 1),
                        )
                ot = stage.tile([P, S], FR, tag="stage", name="ot")
                nc.scalar.activation(ot, po, AF.Relu, bias=pcol(9, m))
                nc.sync.dma_start(outT[m * P : (m + 1) * P, :], ot)
    nc.compile()
    return nc


_NC = None


def _get_nc():
    global _NC
    if _NC is None:
        _NC = build_nc()
    return _NC


def make_in_maps(q, k, v, mask, wq, bq, wk, bk, wv, bv, wo, bo,
                 g1, be1, mm1, mv1, g2, be2, mm2, mv2, g3, be3, mm3, mv3):
    f = np.float32
    r1 = (g1 / np.sqrt(mv1 + BN_EPS)).astype(f)
    r2 = (g2 / np.sqrt(mv2 + BN_EPS)).astype(f)
    r3 = (g3 / np.sqrt(mv3 + BN_EPS)).astype(f)
    scale = f(1.0 / np.sqrt(DP))
    vecs = [
        np.asarray(bq, f), r1 * scale, (be1 - mm1 * r1).astype(f) * scale,
        np.asarray(bk, f), r2, (be2 - mm2 * r2).astype(f),
        np.asarray(bv, f), r3, (be3 - mm3 * r3).astype(f),
        np.asarray(bo, f),
    ]
    # params[p, j*8+m] = vec_j[m*128+p]
    prm = np.stack([x.reshape(CT, P).T for x in vecs], 1).reshape(P, 10 * CT)
    prm = np.ascontiguousarray(prm, f)
    import ml_dtypes
    consts = np.concatenate([np.eye(P, dtype=f), np.ones((P, H), f)], 1)
    shared = {
        "consts": consts,
        "ones_bf": np.ones((P, H), ml_dtypes.bfloat16),
        "wq": np.ascontiguousarray(wq, f),
        "wk": np.ascontiguousarray(wk, f),
        "wv": np.ascontiguousarray(wv, f),
        "wo": np.ascontiguousarray(wo, f),
        "params": prm,
    }
    q, k, v = np.asarray(q, f), np.asarray(k, f), np.asarray(v, f)
    in_maps = []
    for b in range(NCORES):
        mb = (np.asarray(mask[b, 0, 0, :], f) * f(-1e9)).reshape(ST, P).T
        in_maps.append({
            "xq": np.ascontiguousarray(q[b].T),
            "xk": np.ascontiguousarray(k[b].T),
            "xv": np.ascontiguousarray(v[b].T),
            "msk": np.ascontiguousarray(mb, f),
            **shared,
        })
    return in_maps


def kernel(**inputs):
    nc = _get_nc()
    in_maps = make_in_maps(**inputs)
    res = run_bass_kernel_spmd(nc, in_maps, list(range(NCORES))).results
    out = np.stack([res[b]["outT"].T for b in range(NCORES)])
    attn = np.stack(
        [res[b]["attnT"].astype(np.float32) for b in range(NCORES)]
    )
    return out, attn.transpose(0, 1, 3, 2)
